# revision 1
# baseline (speedup 1.0000x reference)
"""Multi-head GAT layer (nn_MultiHeadGraphAttentionLayer) as a Bass/Tile
kernel for one TRN2 chip (8 NeuronCores, SPMD).

Strategy (per core c, owning query rows q in [c*1024, (c+1)*1024)):
  - Transposed orientation: scores S[k, q] with neighbor k on the
    partition axis and query q on the free axis, so the softmax-numerator
    matmul needs no transposes:
        outT[f, q] = sum_k WhO[k, f] * P[k, q]
    with lhsT = WhO = [Wh_h | ones] (fp16) so column 64 accumulates the
    softmax denominator.
  - P = exp(lrelu(logm + dst_h[k] + src_h[q])), logm in {0, -60000} fp16:
    the additive mask commutes with lrelu+exp (exp(lrelu(-huge)) == 0),
    which lets the whole mask+bias+broadcast combine run as ONE DVE
    scalar_tensor_tensor per (k-block, head) tile.
  - lrelu: Prelu on the ACT engine (alpha honored; Lrelu's alpha is NOT)
    for every act_every-th tile group, mul/max on DVE for the rest —
    balances the ACT and DVE engines.  exp: one wide ACT op per group of
    2*kb_group tiles (packed free dim amortizes the per-op overhead).
  - epilogue: reciprocal of the denominator row, broadcast across
    partitions via a K=1 matmul against ones, multiply, elu
    (= exp(min(x,0)) - 1 + max(x,0)), then the fused fc layer as 4
    accumulating K=64 matmuls (per-head chunks of fc_w.T).
  - host side: Wh = h @ W (the sharding hint specifies Wh replicated),
    the O(N) attention dot-products src/dst, fc bias, final gather.
All fp16 on-device elementwise (DVE 2x mode); PSUM accumulation fp32.
"""

import sys
import numpy as np

for _p in ("/opt/trn_rl_repo", "/root/.axon_site/_ro/trn_rl_repo"):
    if _p not in sys.path:
        sys.path.append(_p)

import concourse.bass as bass
import concourse.bacc as bacc
import concourse.mybir as mybir
from concourse import tile
from concourse.bass_utils import run_bass_kernel_spmd

F32 = mybir.dt.float32
F16 = mybir.dt.float16
AF = mybir.ActivationFunctionType
OP = mybir.AluOpType

N = 8192
OUT_F = 64
HEADS = 4
ALPHA = 0.2
NCORES = 8
FO = OUT_F + 1
Q_SLAB = N // NCORES
KB = N // 128
QW = Q_SLAB // 2
MASK_NEG = -60000.0  # fp16-representable; exp(lrelu(x + MASK_NEG)) == 0


def build_kernel(loop_iters=None, act_every=2, kb_group=2):
    G = 2 * kb_group
    nc = bacc.Bacc("TRN2", target_bir_lowering=False, debug=False,
                   num_devices=NCORES)

    who_d = nc.dram_tensor("who", [N, HEADS * FO], F16, kind="ExternalInput")
    dst_d = nc.dram_tensor("dstt", [128, HEADS * KB], F32, kind="ExternalInput")
    src_d = nc.dram_tensor("srcb", [128, HEADS * Q_SLAB], F16,
                           kind="ExternalInput")
    logm_d = nc.dram_tensor("logm", [N, Q_SLAB], F16, kind="ExternalInput")
    fct_d = nc.dram_tensor("fct", [64, HEADS * OUT_F], F32,
                           kind="ExternalInput")
    y_d = nc.dram_tensor("yt", [OUT_F, Q_SLAB], F32, kind="ExternalOutput")

    with tile.TileContext(nc) as tc:
        with (
            tc.tile_pool(name="resident", bufs=1) as res_pool,
            tc.tile_pool(name="logm", bufs=3) as logm_pool,
            tc.tile_pool(name="sbig", bufs=2) as s_pool,
            tc.tile_pool(name="ubig", bufs=2) as u_pool,
            tc.tile_pool(name="vbig", bufs=2) as v_pool,
            tc.tile_pool(name="pbig", bufs=3) as p_pool,
            tc.tile_pool(name="epi", bufs=2) as epi_pool,
            tc.tile_pool(name="hc", bufs=2) as hc_pool,
            tc.tile_pool(name="acc", bufs=6, space=bass.MemorySpace.PSUM)
            as acc_pool,
            tc.tile_pool(name="pmisc", bufs=1, space=bass.MemorySpace.PSUM)
            as pm_pool,
        ):
            who_sb = res_pool.tile([128, KB, HEADS * FO], F16)
            for kb in range(KB):
                nc.sync.dma_start(out=who_sb[:, kb, :],
                                  in_=who_d[kb * 128:(kb + 1) * 128, :])
            dst_sb = res_pool.tile([128, HEADS * KB], F32)
            nc.sync.dma_start(out=dst_sb[:], in_=dst_d[:])
            src_sb = res_pool.tile([128, HEADS * Q_SLAB], F16)
            nc.sync.dma_start(out=src_sb[:], in_=src_d[:])
            fct_sb = res_pool.tile([64, HEADS * OUT_F], F32)
            nc.sync.dma_start(out=fct_sb[:], in_=fct_d[:])
            ones_sb = res_pool.tile([1, 64], F32)
            nc.vector.memset(ones_sb[:], 1.0)
            ysb = res_pool.tile([OUT_F, Q_SLAB], F32)

            def _body():
                hc_tiles = []
                gi = 0
                for sweep in range(2):
                    heads = [2 * sweep, 2 * sweep + 1]
                    accs = {}
                    for hi in range(2):
                        for qh in range(2):
                            accs[(hi, qh)] = acc_pool.tile(
                                [FO, QW], F32, tag="acc", name=f"acc{hi}{qh}")
                    for kb0 in range(0, KB, kb_group):
                        lt = logm_pool.tile([128, kb_group, Q_SLAB], F16)
                        nc.sync.dma_start(
                            out=lt[:],
                            in_=logm_d[kb0 * 128:(kb0 + kb_group) * 128, :]
                            .rearrange("(t p) q -> p t q", p=128))
                        s_big = s_pool.tile([128, G, Q_SLAB], F16)
                        for t in range(kb_group):
                            kb = kb0 + t
                            for hi, h in enumerate(heads):
                                j = 2 * t + hi
                                nc.vector.scalar_tensor_tensor(
                                    s_big[:, j, :], lt[:, t, :],
                                    dst_sb[:, h * KB + kb: h * KB + kb + 1],
                                    src_sb[:, h * Q_SLAB:(h + 1) * Q_SLAB],
                                    op0=OP.add, op1=OP.add)
                        v_big = v_pool.tile([128, G, Q_SLAB], F16)
                        if gi % act_every == 0:
                            # Prelu honors alpha on HW; Lrelu does not.
                            nc.scalar.activation(v_big[:], s_big[:], AF.Prelu,
                                                 alpha=ALPHA)
                        else:
                            u_big = u_pool.tile([128, G, Q_SLAB], F16)
                            nc.vector.tensor_scalar_mul(u_big[:], s_big[:],
                                                        ALPHA)
                            nc.vector.tensor_tensor(v_big[:], s_big[:],
                                                    u_big[:], op=OP.max)
                        gi += 1
                        p_big = p_pool.tile([128, G, Q_SLAB], F16)
                        nc.scalar.activation(p_big[:], v_big[:], AF.Exp)
                        for t in range(kb_group):
                            kb = kb0 + t
                            for hi, h in enumerate(heads):
                                j = 2 * t + hi
                                for qh in range(2):
                                    nc.tensor.matmul(
                                        accs[(hi, qh)][:],
                                        who_sb[:, kb, h * FO:(h + 1) * FO],
                                        p_big[:, j, qh * QW:(qh + 1) * QW],
                                        start=(kb == 0), stop=(kb == KB - 1))
                    for hi, h in enumerate(heads):
                        hc = hc_pool.tile([64, Q_SLAB], F32, tag=f"hc{hi}",
                                          name=f"hc{hi}")
                        hc_tiles.append(hc)
                        for qh in range(2):
                            acc = accs[(hi, qh)]
                            rec = epi_pool.tile([1, QW], F32, tag="rec",
                                                name="rec")
                            nc.vector.reciprocal(rec[:], acc[64:65, :])
                            rb = pm_pool.tile([64, QW], F32, tag="rb",
                                              name="rb")
                            nc.tensor.matmul(rb[:], ones_sb[:], rec[:],
                                             start=True, stop=True)
                            rb_sb = epi_pool.tile([64, QW], F32, tag="rb_sb",
                                                  name="rb_sb")
                            nc.scalar.copy(rb_sb[:], rb[:])
                            hcs = hc[:, qh * QW:(qh + 1) * QW]
                            nc.vector.tensor_tensor(hcs, acc[0:64, :],
                                                    rb_sb[:], op=OP.mult)
                            t1 = epi_pool.tile([64, QW], F32, tag="t1",
                                               name="t1")
                            nc.vector.tensor_scalar_min(t1[:], hcs, 0.0)
                            t2 = epi_pool.tile([64, QW], F32, tag="t2",
                                               name="t2")
                            nc.scalar.activation(t2[:], t1[:], AF.Exp)
                            t3 = epi_pool.tile([64, QW], F32, tag="t3",
                                               name="t3")
                            nc.vector.tensor_scalar_max(t3[:], hcs, 0.0)
                            # elu = (exp(min(x,0)) - 1) + max(x,0)
                            nc.vector.scalar_tensor_tensor(
                                hcs, t2[:], -1.0, t3[:], op0=OP.add,
                                op1=OP.add)
                for qh in range(2):
                    y_ps = pm_pool.tile([OUT_F, QW], F32, tag="y_ps",
                                        name="y_ps")
                    for h in range(HEADS):
                        nc.tensor.matmul(
                            y_ps[:],
                            fct_sb[:, h * OUT_F:(h + 1) * OUT_F],
                            hc_tiles[h][:, qh * QW:(qh + 1) * QW],
                            start=(h == 0), stop=(h == HEADS - 1))
                    nc.scalar.copy(ysb[:, qh * QW:(qh + 1) * QW], y_ps[:])

            if loop_iters is not None:
                with tc.For_i(0, loop_iters, 1):
                    _body()
            else:
                _body()
            nc.sync.dma_start(out=y_d[:], in_=ysb[:])
    nc.finalize()
    return nc


def host_prep(h, adj, W, a1, a2, fc_w):
    h = np.asarray(h, np.float32)
    W = np.asarray(W, np.float32)
    Wh = np.einsum('ni,hio->hno', h, W, optimize=True).astype(np.float32)
    src = np.einsum('hno,ho->hn', Wh, np.asarray(a1, np.float32))
    dst = np.einsum('hno,ho->hn', Wh, np.asarray(a2, np.float32))

    who = np.empty((N, HEADS * FO), np.float16)
    for hh in range(HEADS):
        who[:, hh * FO:hh * FO + OUT_F] = Wh[hh]
        who[:, hh * FO + OUT_F] = 1.0

    dstt = np.ascontiguousarray(
        dst.reshape(HEADS, KB, 128).transpose(2, 0, 1)
        .reshape(128, HEADS * KB))
    fct = np.ascontiguousarray(
        np.asarray(fc_w, np.float32).T.reshape(HEADS, 64, OUT_F)
        .transpose(1, 0, 2).reshape(64, HEADS * OUT_F))

    lut = np.array([MASK_NEG, 0.0], np.float16)
    adjT = np.ascontiguousarray(np.asarray(adj).T)

    in_maps = []
    for c in range(NCORES):
        q0 = c * Q_SLAB
        srcb = np.broadcast_to(
            np.ascontiguousarray(src[:, q0:q0 + Q_SLAB]).reshape(1, -1),
            (128, HEADS * Q_SLAB)).astype(np.float16)
        logm = lut[(adjT[:, q0:q0 + Q_SLAB] > 0).astype(np.int8)]
        in_maps.append({
            "who": who, "dstt": dstt, "fct": fct,
            "srcb": np.ascontiguousarray(srcb),
            "logm": np.ascontiguousarray(logm),
        })
    return in_maps


_NC_CACHE = {}


def kernel(h, adj, W, a1, a2, fc_w, fc_b):
    if "nc" not in _NC_CACHE:
        _NC_CACHE["nc"] = build_kernel()
    nc = _NC_CACHE["nc"]
    in_maps = host_prep(h, adj, W, a1, a2, fc_w)
    res = run_bass_kernel_spmd(nc, in_maps, list(range(NCORES)))
    yt = np.concatenate([res.results[c]["yt"] for c in range(NCORES)], axis=1)
    return (yt.T + np.asarray(fc_b, np.float32)[None, :]).astype(np.float32)


# revision 2
# speedup vs baseline: 1.4796x; 1.4796x over previous
"""Multi-head GAT layer (nn_MultiHeadGraphAttentionLayer) as a Bass/Tile
kernel for one TRN2 chip (8 NeuronCores, SPMD).

Strategy (per core c, owning query rows q in [c*1024, (c+1)*1024)):
  - Transposed orientation: scores S[k, q] with neighbor k on the
    partition axis and query q on the free axis, so the softmax-numerator
    matmul needs no transposes:
        outT[f, q] = sum_k WhO[k, f] * P[k, q]
    with lhsT = WhO = [Wh_h | ones] (fp16) so column 64 accumulates the
    softmax denominator.
  - P = exp(lrelu(logm + dst_h[k] + src_h[q])), logm in {0, -60000} fp16:
    the additive mask commutes with lrelu+exp (exp(lrelu(-huge)) == 0),
    which lets the whole mask+bias+broadcast combine run as ONE DVE
    scalar_tensor_tensor per (k-block, head) tile.
  - lrelu: Prelu on the ACT engine (alpha honored; Lrelu's alpha is NOT)
    for every act_every-th tile group, mul/max on DVE for the rest —
    balances the ACT and DVE engines.  exp: one wide ACT op per group of
    2*kb_group tiles (packed free dim amortizes the per-op overhead).
  - epilogue: reciprocal of the denominator row, broadcast across
    partitions via a K=1 matmul against ones, multiply, elu
    (= exp(min(x,0)) - 1 + max(x,0)), then the fused fc layer as 4
    accumulating K=64 matmuls (per-head chunks of fc_w.T).
  - host side: Wh = h @ W (the sharding hint specifies Wh replicated),
    the O(N) attention dot-products src/dst, fc bias, final gather.
All fp16 on-device elementwise (DVE 2x mode); PSUM accumulation fp32.
"""

import sys
import numpy as np

for _p in ("/opt/trn_rl_repo", "/root/.axon_site/_ro/trn_rl_repo"):
    if _p not in sys.path:
        sys.path.append(_p)

import concourse.bass as bass
import concourse.bacc as bacc
import concourse.mybir as mybir
from concourse import tile
from concourse.bass_utils import run_bass_kernel_spmd

F32 = mybir.dt.float32
F16 = mybir.dt.float16
AF = mybir.ActivationFunctionType
OP = mybir.AluOpType

N = 8192
OUT_F = 64
HEADS = 4
ALPHA = 0.2
NCORES = 8
FO = OUT_F + 1
Q_SLAB = N // NCORES
KB = N // 128
QW = Q_SLAB // 2
MASK_NEG = -60000.0  # fp16-representable; exp(lrelu(x + MASK_NEG)) == 0


def build_kernel(loop_iters=None, act_every=3, kb_group=2):
    G = 2 * kb_group
    nc = bacc.Bacc("TRN2", target_bir_lowering=False, debug=False,
                   num_devices=NCORES)

    who_d = nc.dram_tensor("who", [N, HEADS * FO], F16, kind="ExternalInput")
    dst_d = nc.dram_tensor("dstt", [128, HEADS * KB], F32, kind="ExternalInput")
    src_d = nc.dram_tensor("srcb", [128, HEADS * Q_SLAB], F16,
                           kind="ExternalInput")
    logm_d = nc.dram_tensor("logm", [N, Q_SLAB], F16, kind="ExternalInput")
    fct_d = nc.dram_tensor("fct", [64, HEADS * OUT_F], F32,
                           kind="ExternalInput")
    y_d = nc.dram_tensor("yt", [OUT_F, Q_SLAB], F32, kind="ExternalOutput")

    with tile.TileContext(nc) as tc:
        with (
            tc.tile_pool(name="resident", bufs=1) as res_pool,
            tc.tile_pool(name="logm", bufs=3) as logm_pool,
            tc.tile_pool(name="sbig", bufs=2) as s_pool,
            tc.tile_pool(name="ubig", bufs=2) as u_pool,
            tc.tile_pool(name="vbig", bufs=2) as v_pool,
            tc.tile_pool(name="pbig", bufs=3) as p_pool,
            tc.tile_pool(name="epi", bufs=2) as epi_pool,
            tc.tile_pool(name="hc", bufs=2) as hc_pool,
            tc.tile_pool(name="acc", bufs=6, space=bass.MemorySpace.PSUM)
            as acc_pool,
            tc.tile_pool(name="pmisc", bufs=1, space=bass.MemorySpace.PSUM)
            as pm_pool,
        ):
            who_sb = res_pool.tile([128, KB, HEADS * FO], F16)
            for kb in range(KB):
                nc.sync.dma_start(out=who_sb[:, kb, :],
                                  in_=who_d[kb * 128:(kb + 1) * 128, :])
            dst_sb = res_pool.tile([128, HEADS * KB], F32)
            nc.sync.dma_start(out=dst_sb[:], in_=dst_d[:])
            src_sb = res_pool.tile([128, HEADS * Q_SLAB], F16)
            nc.sync.dma_start(out=src_sb[:], in_=src_d[:])
            fct_sb = res_pool.tile([64, HEADS * OUT_F], F32)
            nc.sync.dma_start(out=fct_sb[:], in_=fct_d[:])
            ones_sb = res_pool.tile([1, 64], F32)
            nc.vector.memset(ones_sb[:], 1.0)
            ysb = res_pool.tile([OUT_F, Q_SLAB], F32)

            def _body():
                hc_tiles = []
                gi = 0
                for sweep in range(2):
                    heads = [2 * sweep, 2 * sweep + 1]
                    accs = {}
                    for hi in range(2):
                        for qh in range(2):
                            accs[(hi, qh)] = acc_pool.tile(
                                [FO, QW], F32, tag="acc", name=f"acc{hi}{qh}")
                    for kb0 in range(0, KB, kb_group):
                        lt = logm_pool.tile([128, kb_group, Q_SLAB], F16)
                        nc.sync.dma_start(
                            out=lt[:],
                            in_=logm_d[kb0 * 128:(kb0 + kb_group) * 128, :]
                            .rearrange("(t p) q -> p t q", p=128))
                        s_big = s_pool.tile([128, G, Q_SLAB], F16)
                        for t in range(kb_group):
                            kb = kb0 + t
                            for hi, h in enumerate(heads):
                                j = 2 * t + hi
                                nc.vector.scalar_tensor_tensor(
                                    s_big[:, j, :], lt[:, t, :],
                                    dst_sb[:, h * KB + kb: h * KB + kb + 1],
                                    src_sb[:, h * Q_SLAB:(h + 1) * Q_SLAB],
                                    op0=OP.add, op1=OP.add)
                        v_big = v_pool.tile([128, G, Q_SLAB], F16)
                        if gi % act_every == 0:
                            # Prelu honors alpha on HW; Lrelu does not.
                            nc.scalar.activation(v_big[:], s_big[:], AF.Prelu,
                                                 alpha=ALPHA)
                        else:
                            u_big = u_pool.tile([128, G, Q_SLAB], F16)
                            nc.vector.tensor_scalar_mul(u_big[:], s_big[:],
                                                        ALPHA)
                            nc.vector.tensor_tensor(v_big[:], s_big[:],
                                                    u_big[:], op=OP.max)
                        gi += 1
                        p_big = p_pool.tile([128, G, Q_SLAB], F16)
                        nc.scalar.activation(p_big[:], v_big[:], AF.Exp)
                        for t in range(kb_group):
                            kb = kb0 + t
                            for hi, h in enumerate(heads):
                                j = 2 * t + hi
                                for qh in range(2):
                                    nc.tensor.matmul(
                                        accs[(hi, qh)][:],
                                        who_sb[:, kb, h * FO:(h + 1) * FO],
                                        p_big[:, j, qh * QW:(qh + 1) * QW],
                                        start=(kb == 0), stop=(kb == KB - 1))
                    for hi, h in enumerate(heads):
                        hc = hc_pool.tile([64, Q_SLAB], F32, tag=f"hc{hi}",
                                          name=f"hc{hi}")
                        hc_tiles.append(hc)
                        for qh in range(2):
                            acc = accs[(hi, qh)]
                            rec = epi_pool.tile([1, QW], F32, tag="rec",
                                                name="rec")
                            nc.vector.reciprocal(rec[:], acc[64:65, :])
                            rb = pm_pool.tile([64, QW], F32, tag="rb",
                                              name="rb")
                            nc.tensor.matmul(rb[:], ones_sb[:], rec[:],
                                             start=True, stop=True)
                            rb_sb = epi_pool.tile([64, QW], F32, tag="rb_sb",
                                                  name="rb_sb")
                            nc.scalar.copy(rb_sb[:], rb[:])
                            hcs = hc[:, qh * QW:(qh + 1) * QW]
                            nc.vector.tensor_tensor(hcs, acc[0:64, :],
                                                    rb_sb[:], op=OP.mult)
                            t1 = epi_pool.tile([64, QW], F32, tag="t1",
                                               name="t1")
                            nc.vector.tensor_scalar_min(t1[:], hcs, 0.0)
                            t2 = epi_pool.tile([64, QW], F32, tag="t2",
                                               name="t2")
                            nc.scalar.activation(t2[:], t1[:], AF.Exp)
                            t3 = epi_pool.tile([64, QW], F32, tag="t3",
                                               name="t3")
                            nc.vector.tensor_scalar_max(t3[:], hcs, 0.0)
                            # elu = (exp(min(x,0)) - 1) + max(x,0)
                            nc.vector.scalar_tensor_tensor(
                                hcs, t2[:], -1.0, t3[:], op0=OP.add,
                                op1=OP.add)
                for qh in range(2):
                    y_ps = pm_pool.tile([OUT_F, QW], F32, tag="y_ps",
                                        name="y_ps")
                    for h in range(HEADS):
                        nc.tensor.matmul(
                            y_ps[:],
                            fct_sb[:, h * OUT_F:(h + 1) * OUT_F],
                            hc_tiles[h][:, qh * QW:(qh + 1) * QW],
                            start=(h == 0), stop=(h == HEADS - 1))
                    nc.scalar.copy(ysb[:, qh * QW:(qh + 1) * QW], y_ps[:])

            if loop_iters is not None:
                with tc.For_i(0, loop_iters, 1):
                    _body()
            else:
                _body()
            nc.sync.dma_start(out=y_d[:], in_=ysb[:])
    nc.finalize()
    return nc


def host_prep(h, adj, W, a1, a2, fc_w):
    h = np.asarray(h, np.float32)
    W = np.asarray(W, np.float32)
    Wh = np.einsum('ni,hio->hno', h, W, optimize=True).astype(np.float32)
    src = np.einsum('hno,ho->hn', Wh, np.asarray(a1, np.float32))
    dst = np.einsum('hno,ho->hn', Wh, np.asarray(a2, np.float32))

    who = np.empty((N, HEADS * FO), np.float16)
    for hh in range(HEADS):
        who[:, hh * FO:hh * FO + OUT_F] = Wh[hh]
        who[:, hh * FO + OUT_F] = 1.0

    dstt = np.ascontiguousarray(
        dst.reshape(HEADS, KB, 128).transpose(2, 0, 1)
        .reshape(128, HEADS * KB))
    fct = np.ascontiguousarray(
        np.asarray(fc_w, np.float32).T.reshape(HEADS, 64, OUT_F)
        .transpose(1, 0, 2).reshape(64, HEADS * OUT_F))

    lut = np.array([MASK_NEG, 0.0], np.float16)
    adjT = np.ascontiguousarray(np.asarray(adj).T)

    in_maps = []
    for c in range(NCORES):
        q0 = c * Q_SLAB
        srcb = np.broadcast_to(
            np.ascontiguousarray(src[:, q0:q0 + Q_SLAB]).reshape(1, -1),
            (128, HEADS * Q_SLAB)).astype(np.float16)
        logm = lut[(adjT[:, q0:q0 + Q_SLAB] > 0).astype(np.int8)]
        in_maps.append({
            "who": who, "dstt": dstt, "fct": fct,
            "srcb": np.ascontiguousarray(srcb),
            "logm": np.ascontiguousarray(logm),
        })
    return in_maps


_NC_CACHE = {}


def kernel(h, adj, W, a1, a2, fc_w, fc_b):
    if "nc" not in _NC_CACHE:
        _NC_CACHE["nc"] = build_kernel()
    nc = _NC_CACHE["nc"]
    in_maps = host_prep(h, adj, W, a1, a2, fc_w)
    res = run_bass_kernel_spmd(nc, in_maps, list(range(NCORES)))
    yt = np.concatenate([res.results[c]["yt"] for c in range(NCORES)], axis=1)
    return (yt.T + np.asarray(fc_b, np.float32)[None, :]).astype(np.float32)
